# revision 1
# baseline (speedup 1.0000x reference)
"""CrossSourceMHA Trainium2 kernel.

Full inputs -> full output; shards batch N=8 across 8 NeuronCores (1 batch
element per core). Per core, channels C=128 live on SBUF partitions.

Per-core pipeline:
  - avgpool(k_src), avgpool(v_src) as 2x2 SUMS (1/4 folded into GN affine)
  - GroupNorm chains folded into conv weights: bn_stats per channel ->
    group reduce/broadcast via tiny indicator matmuls -> per-channel affine
    (A, B) -> W' = W^T * A (rows), bias' = W @ B + b.  The stacked double-GN
    of the K/V paths is chained analytically (no second data pass).
  - convs as single fp32r matmuls (channels on partitions)
  - V^T produced directly by the V-conv in transposed form (lhsT = pooled V)
  - scores S^T = K^T Q per head via K=32 row-tiled fp32r matmuls
  - exp on ACT (PSUM -> bf16 SBUF)
  - AV and softmax denominator via col-tiled M=32 bf16 matmuls
    (den: ones lhsT -> every partition of the head slot holds den)
  - normalize with reciprocal_approx_fast, add V-path bias (deferred through
    the attention: softmax(.) @ (v + b) = softmax(.) @ v + b), Wp conv, + bp

_build(reps=N) emits the whole pipeline N times (benchmarking: slope between
rep counts isolates HW time from axon dispatch overhead). reps=1 for grading.
"""

import numpy as np

import concourse.bacc as bacc
import concourse.bass as bass
import concourse.mybir as mybir
import concourse.tile as tile
from concourse.bass_utils import run_bass_kernel_spmd

B = 8
C = 128
H = 64
W = 64
S = H * W          # 4096 q positions
T = S // 4         # 1024 kv positions after 2x2 pool
HEADS = 4
D = C // HEADS     # 32
GROUPS = 32
GSZ = C // GROUPS  # 4 channels per group
EPS = 1e-5
SCALE = D ** -0.5

NQT = 8            # q tiles of 512
QT = S // NQT      # 512
NKT = T // 128     # 8 lk tiles of 128

FP32 = mybir.dt.float32
FP32R = mybir.dt.float32r
BF16 = mybir.dt.bfloat16
AOP = mybir.AluOpType
AF = mybir.ActivationFunctionType

_CACHE = {}


def _vec_in(nc, name):
    return nc.dram_tensor(name, [C, 1], FP32, kind="ExternalInput")


def _build(reps=1):
    nc = bacc.Bacc()

    qs_d = nc.dram_tensor("qs", [C, S], FP32, kind="ExternalInput")
    ks_d = nc.dram_tensor("ks", [C, S], FP32, kind="ExternalInput")
    vs_d = nc.dram_tensor("vs", [C, S], FP32, kind="ExternalInput")
    wqt_d = nc.dram_tensor("wqt", [C, C], FP32, kind="ExternalInput")  # Wq.T*SCALE
    wkt_d = nc.dram_tensor("wkt", [C, C], FP32, kind="ExternalInput")  # Wk.T
    wvt_d = nc.dram_tensor("wvt", [C, C], FP32, kind="ExternalInput")  # Wv.T
    wpt_d = nc.dram_tensor("wpt", [C, C], FP32, kind="ExternalInput")  # Wp.T
    bq_d = _vec_in(nc, "bqv")   # bq*SCALE
    bk_d = _vec_in(nc, "bkv")
    bv_d = _vec_in(nc, "bvv")
    bp_d = _vec_in(nc, "bpv")
    gnq_d, bnq_d = _vec_in(nc, "gnq"), _vec_in(nc, "bnq")
    gnk_d, bnk_d = _vec_in(nc, "gnk"), _vec_in(nc, "bnk")
    gnv_d, bnv_d = _vec_in(nc, "gnv"), _vec_in(nc, "bnv")
    gsrk_d, bsrk_d = _vec_in(nc, "gsrk"), _vec_in(nc, "bsrk")
    gsrv_d, bsrv_d = _vec_in(nc, "gsrv"), _vec_in(nc, "bsrv")
    g_d = nc.dram_tensor("gmat", [C, GROUPS], FP32, kind="ExternalInput")
    gt_d = nc.dram_tensor("gtmat", [GROUPS, C], FP32, kind="ExternalInput")

    out_d = nc.dram_tensor("out", [C, S], FP32, kind="ExternalOutput")

    with tile.TileContext(nc) as tc:
        with (
            tc.tile_pool(name="persist", bufs=1) as pp,
            tc.tile_pool(name="ptpool", bufs=24) as ptp,
            tc.tile_pool(name="opool", bufs=3) as op,
        ):
            # ---- one-time: weight/constant loads ----
            def load(d, shape, tag):
                t = pp.tile(shape, FP32, tag=tag, name=f"ld_{tag}")
                nc.sync.dma_start(out=t, in_=d[:, :])
                return t

            wqt = load(wqt_d, [C, C], "wqt")
            wkt = load(wkt_d, [C, C], "wkt")
            wvt = load(wvt_d, [C, C], "wvt")
            wpt = load(wpt_d, [C, C], "wpt")
            bq1 = load(bq_d, [C, 1], "bq1")
            bk1 = load(bk_d, [C, 1], "bk1")
            bv1 = load(bv_d, [C, 1], "bv1")
            bp1 = load(bp_d, [C, 1], "bp1")
            gnq = load(gnq_d, [C, 1], "gnq")
            bnq = load(bnq_d, [C, 1], "bnq")
            gnk = load(gnk_d, [C, 1], "gnk")
            bnk = load(bnk_d, [C, 1], "bnk")
            gnv = load(gnv_d, [C, 1], "gnv")
            bnv = load(bnv_d, [C, 1], "bnv")
            gsrk = load(gsrk_d, [C, 1], "gsrk")
            bsrk = load(bsrk_d, [C, 1], "bsrk")
            gsrv = load(gsrv_d, [C, 1], "gsrv")
            bsrv = load(bsrv_d, [C, 1], "bsrv")
            g_sb = load(g_d, [C, GROUPS], "gmat")
            gt_sb = load(gt_d, [GROUPS, C], "gtmat")

            eps_sb = pp.tile([C, 1], FP32, tag="eps", name="eps_sb")
            nc.vector.memset(eps_sb[:, :], EPS)
            ones_f = pp.tile([C, D], FP32, tag="ones_f", name="ones_f")
            nc.vector.memset(ones_f[:, :], 1.0)
            ones_bf = pp.tile([C, D], BF16, tag="ones_bf", name="ones_bf")
            nc.vector.tensor_copy(ones_bf[:, :], ones_f[:, :])

            wp_r = pp.tile([C, C], FP32R, tag="wp_r", name="wp_r")
            nc.vector.tensor_copy(wp_r[:, :], wpt[:, :])

            for rep in range(reps):
                r = f"r{rep}"

                def _group_stats(pspool, cstats, path):
                    """cstats [128,2] = per-channel [mean, E[x^2]] ->
                    [128,2] per-channel [mu_group, rstd_group]."""
                    grp_ps = pspool.tile([GROUPS, 2], FP32, tag="small", name=f"gps_{path}_{r}")
                    nc.tensor.matmul(grp_ps[:, :], g_sb[:, :], cstats[:, :], start=True, stop=True)
                    grp = pp.tile([GROUPS, 2], FP32, tag=f"grp_{path}", name=f"grp_{path}_{r}")
                    nc.vector.tensor_scalar_mul(grp[:, :], grp_ps[:, :], 1.0 / GSZ)
                    var = pp.tile([GROUPS, 1], FP32, tag=f"var_{path}", name=f"var_{path}_{r}")
                    nc.vector.tensor_mul(var[:, :], grp[:, 0:1], grp[:, 0:1])
                    nc.vector.tensor_tensor(var[:, :], grp[:, 1:2], var[:, :], AOP.subtract)
                    sd = pp.tile([GROUPS, 1], FP32, tag=f"sd_{path}", name=f"sd_{path}_{r}")
                    nc.scalar.activation(sd[:, :], var[:, :], AF.Sqrt, bias=eps_sb[:GROUPS, :])
                    nc.vector.reciprocal(grp[:, 1:2], sd[:, :])
                    bc_ps = pspool.tile([C, 2], FP32, tag="small", name=f"bps_{path}_{r}")
                    nc.tensor.matmul(bc_ps[:, :], gt_sb[:, :], grp[:, :], start=True, stop=True)
                    chan = pp.tile([C, 2], FP32, tag=f"chan_{path}", name=f"chan_{path}_{r}")
                    nc.vector.tensor_copy(chan[:, :], bc_ps[:, :])
                    return chan

                def _affine(chan, gamma, beta, path):
                    a = pp.tile([C, 1], FP32, tag=f"a_{path}", name=f"a_{path}_{r}")
                    nc.vector.tensor_mul(a[:, :], chan[:, 1:2], gamma[:, :])
                    b = pp.tile([C, 1], FP32, tag=f"b_{path}", name=f"b_{path}_{r}")
                    nc.vector.tensor_mul(b[:, :], chan[:, 0:1], a[:, :])
                    nc.vector.tensor_tensor(b[:, :], beta[:, :], b[:, :], AOP.subtract)
                    return a, b

                # ---------------- stage 0: data loads + pooling ----------------
                q_f = pp.tile([C, S], FP32, tag="q_f", name=f"q_f_{r}")
                nc.sync.dma_start(out=q_f, in_=qs_d[:, :])
                q_sb = pp.tile([C, S], FP32R, tag="q", name=f"q_sb_{r}")
                nc.vector.tensor_copy(q_sb[:, :], q_f[:, :])

                ksum = pp.tile([C, T], FP32R, tag="ksum", name=f"ksum_{r}")
                vsum = pp.tile([C, T], FP32R, tag="vsum", name=f"vsum_{r}")
                with tc.tile_pool(name=f"poolscratch_{r}", bufs=1) as sc:
                    for src_d, dst, tag in ((ks_d, ksum, "k"), (vs_d, vsum, "v")):
                        raw = sc.tile([C, S], FP32, tag=f"raw{tag}", name=f"raw{tag}_{r}")
                        nc.sync.dma_start(out=raw, in_=src_d[:, :])
                        rw = raw[:, :].rearrange("p (x two) -> p x two", two=2)
                        wsum = sc.tile([C, S // 2], FP32, tag=f"w{tag}", name=f"w{tag}_{r}")
                        nc.vector.tensor_add(wsum[:, :], rw[:, :, 0], rw[:, :, 1])
                        hw = wsum[:, :].rearrange(
                            "p (h two w) -> p h two w", two=2, w=W // 2
                        )
                        nc.vector.tensor_add(
                            dst[:, :].rearrange("p (h w) -> p h w", w=W // 2),
                            hw[:, :, 0, :],
                            hw[:, :, 1, :],
                        )

                # ---------------- stage 1: stats + weight folds ----------------
                with tc.tile_pool(name=f"ps_small_{r}", bufs=2, space="PSUM") as pss:
                    qstats = pp.tile([C, NQT, 6], FP32, tag="qstats", name=f"qstats_{r}")
                    qv = q_sb[:, :].bitcast(FP32).rearrange("p (n f) -> p n f", f=QT)
                    for n in range(NQT):
                        nc.vector.bn_stats(qstats[:, n, :], qv[:, n, :])
                    mvq = pp.tile([C, 2], FP32, tag="mvq", name=f"mvq_{r}")
                    nc.vector.bn_aggr(mvq[:, :], qstats[:, :, :])
                    cs_q = pp.tile([C, 2], FP32, tag="cs_q", name=f"cs_q_{r}")
                    nc.vector.tensor_copy(cs_q[:, 0:1], mvq[:, 0:1])
                    nc.vector.tensor_mul(cs_q[:, 1:2], mvq[:, 0:1], mvq[:, 0:1])
                    nc.vector.tensor_add(cs_q[:, 1:2], cs_q[:, 1:2], mvq[:, 1:2])
                    chan_q = _group_stats(pss, cs_q, "q")
                    a_q, b_q = _affine(chan_q, gnq, bnq, "q")

                    def fold_weight(wt, a, path):
                        wr = pp.tile([C, C], FP32R, tag=f"wr_{path}", name=f"wr_{path}_{r}")
                        nc.vector.tensor_scalar_mul(wr[:, :], wt[:, :], a[:, :])
                        return wr

                    def fold_bias(wt, bvec, baddvec, path):
                        ps = pss.tile([C, 1], FP32, tag="small", name=f"fb_{path}_{r}")
                        nc.tensor.matmul(ps[:, :], wt[:, :], bvec[:, :], start=True, stop=True)
                        out = pp.tile([C, 1], FP32, tag=f"bias_{path}", name=f"bias_{path}_{r}")
                        nc.vector.tensor_add(out[:, :], ps[:, :], baddvec[:, :])
                        return out

                    wq_r = fold_weight(wqt, a_q, "wq")
                    bias_q = fold_bias(wqt, b_q, bq1, "wq")

                    def kv_path(sumtile, gamma1, beta1, gamma2, beta2, wt, bvec, path):
                        sstats = pp.tile(
                            [C, T // QT, 6], FP32, tag=f"sst_{path}", name=f"sst_{path}_{r}"
                        )
                        sv = sumtile[:, :].bitcast(FP32).rearrange(
                            "p (n f) -> p n f", f=QT
                        )
                        for n in range(T // QT):
                            nc.vector.bn_stats(sstats[:, n, :], sv[:, n, :])
                        mvs = pp.tile([C, 2], FP32, tag=f"mvs_{path}", name=f"mvs_{path}_{r}")
                        nc.vector.bn_aggr(mvs[:, :], sstats[:, :, :])
                        raw = pp.tile([C, 2], FP32, tag=f"rst_{path}", name=f"rst_{path}_{r}")
                        nc.vector.tensor_copy(raw[:, 0:1], mvs[:, 0:1])
                        nc.vector.tensor_mul(raw[:, 1:2], mvs[:, 0:1], mvs[:, 0:1])
                        nc.vector.tensor_add(raw[:, 1:2], raw[:, 1:2], mvs[:, 1:2])
                        pst = pp.tile([C, 2], FP32, tag=f"pst_{path}", name=f"pst_{path}_{r}")
                        nc.vector.tensor_scalar_mul(pst[:, 0:1], raw[:, 0:1], 0.25)
                        nc.vector.tensor_scalar_mul(pst[:, 1:2], raw[:, 1:2], 0.0625)
                        chan1 = _group_stats(pss, pst, f"{path}1")
                        a1, b1 = _affine(chan1, gamma1, beta1, f"{path}1")
                        A1 = pp.tile([C, 1], FP32, tag=f"A1_{path}", name=f"A1_{path}_{r}")
                        nc.vector.tensor_scalar_mul(A1[:, :], a1[:, :], 0.25)
                        yst = pp.tile([C, 2], FP32, tag=f"yst_{path}", name=f"yst_{path}_{r}")
                        nc.vector.tensor_mul(yst[:, 0:1], A1[:, :], raw[:, 0:1])
                        nc.vector.tensor_add(yst[:, 0:1], yst[:, 0:1], b1[:, :])
                        t1 = pp.tile([C, 1], FP32, tag=f"t1_{path}", name=f"t1_{path}_{r}")
                        nc.vector.tensor_mul(t1[:, :], A1[:, :], A1[:, :])
                        nc.vector.tensor_mul(t1[:, :], t1[:, :], raw[:, 1:2])
                        t2 = pp.tile([C, 1], FP32, tag=f"t2_{path}", name=f"t2_{path}_{r}")
                        nc.vector.tensor_mul(t2[:, :], A1[:, :], b1[:, :])
                        nc.vector.tensor_mul(t2[:, :], t2[:, :], raw[:, 0:1])
                        nc.vector.tensor_scalar_mul(t2[:, :], t2[:, :], 2.0)
                        nc.vector.tensor_add(t1[:, :], t1[:, :], t2[:, :])
                        nc.vector.tensor_mul(t2[:, :], b1[:, :], b1[:, :])
                        nc.vector.tensor_add(yst[:, 1:2], t1[:, :], t2[:, :])
                        chan2 = _group_stats(pss, yst, f"{path}2")
                        a2, b2 = _affine(chan2, gamma2, beta2, f"{path}2")
                        A = pp.tile([C, 1], FP32, tag=f"A_{path}", name=f"A_{path}_{r}")
                        nc.vector.tensor_mul(A[:, :], A1[:, :], a2[:, :])
                        Bv = pp.tile([C, 1], FP32, tag=f"Bf_{path}", name=f"Bf_{path}_{r}")
                        nc.vector.tensor_mul(Bv[:, :], b1[:, :], a2[:, :])
                        nc.vector.tensor_add(Bv[:, :], Bv[:, :], b2[:, :])
                        wr = fold_weight(wt, A, path)
                        bias = fold_bias(wt, Bv, bvec, path)
                        return wr, bias

                    wk_r, bias_k = kv_path(ksum, gsrk, bsrk, gnk, bnk, wkt, bk1, "k")
                    wv_r, bias_v = kv_path(vsum, gsrv, bsrv, gnv, bnv, wvt, bv1, "v")

                # ---------------- stage 2: convs ----------------
                q_proj = pp.tile([C, S], FP32R, tag="q_proj", name=f"q_proj_{r}")
                k_proj = pp.tile([C, T], FP32R, tag="k_proj", name=f"k_proj_{r}")
                vt_bf = pp.tile([C, NKT, C], BF16, tag="vt_bf", name=f"vt_bf_{r}")
                with tc.tile_pool(name=f"ps_conv_{r}", bufs=2, space="PSUM") as psc:
                    for i in range(NQT):
                        cp = psc.tile([C, QT], FP32, tag="conv", name=f"qc{i}_{r}")
                        nc.tensor.matmul(
                            cp[:, :], wq_r[:, :], q_sb[:, i * QT : (i + 1) * QT],
                            start=True, stop=True,
                        )
                        nc.vector.tensor_scalar(
                            q_proj[:, i * QT : (i + 1) * QT], cp[:, :],
                            bias_q[:, :], None, AOP.add,
                        )
                    for i in range(T // QT):
                        cp = psc.tile([C, QT], FP32, tag="conv", name=f"kc{i}_{r}")
                        nc.tensor.matmul(
                            cp[:, :], wk_r[:, :], ksum[:, i * QT : (i + 1) * QT],
                            start=True, stop=True,
                        )
                        nc.vector.tensor_scalar(
                            k_proj[:, i * QT : (i + 1) * QT], cp[:, :],
                            bias_k[:, :], None, AOP.add,
                        )
                    for t in range(NKT):
                        cp = psc.tile([C, C], FP32, tag="vt", name=f"vtc{t}_{r}")
                        nc.tensor.matmul(
                            cp[:, :], vsum[:, t * C : (t + 1) * C], wv_r[:, :],
                            start=True, stop=True,
                        )
                        nc.vector.tensor_copy(vt_bf[:, t, :], cp[:, :])

                # ---------------- stage 3+4: attention ----------------
                with (
                    tc.tile_pool(name=f"ps_st_{r}", bufs=2, space="PSUM") as ps_st,
                    tc.tile_pool(name=f"ps_av_{r}", bufs=1, space="PSUM") as ps_av,
                    tc.tile_pool(name=f"ps_den_{r}", bufs=1, space="PSUM") as ps_den,
                    tc.tile_pool(name=f"ps_wp_{r}", bufs=2, space="PSUM") as ps_wp,
                ):
                    for i in range(NQT):
                        qsl = q_proj[:, i * QT : (i + 1) * QT]
                        pt = {}
                        for h in range(HEADS):
                            ksl = k_proj[32 * h : 32 * h + 32, :]
                            qh = qsl[32 * h : 32 * h + 32, :]
                            for tg in range(NKT // 2):
                                st = ps_st.tile(
                                    [C, 2, QT], FP32, tag="st", name=f"st{i}_{h}_{tg}_{r}"
                                )
                                for ts in range(2):
                                    t = 2 * tg + ts
                                    nc.tensor.matmul(
                                        st[:, ts, :],
                                        ksl[:, t * C : (t + 1) * C],
                                        qh,
                                        start=True, stop=True,
                                        tile_position=(32 * h, 0),
                                    )
                                p = ptp.tile(
                                    [C, 2, QT], BF16, tag="pt", name=f"pt{i}_{h}_{tg}_{r}"
                                )
                                nc.scalar.activation(p[:, :, :], st[:, :, :], AF.Exp)
                                pt[(h, tg)] = p

                        av = ps_av.tile([C, QT], FP32, tag="av", name=f"av{i}_{r}")
                        den = ps_den.tile([C, QT], FP32, tag="den", name=f"den{i}_{r}")
                        for t in range(NKT):
                            for h in range(HEADS):
                                nc.tensor.matmul(
                                    av[32 * h : 32 * h + 32, :],
                                    vt_bf[:, t, 32 * h : 32 * h + 32],
                                    pt[(h, t // 2)][:, t % 2, :],
                                    start=(t == 0), stop=(t == NKT - 1),
                                    tile_position=(0, 32 * h),
                                )
                            for h in range(HEADS):
                                nc.tensor.matmul(
                                    den[32 * h : 32 * h + 32, :],
                                    ones_bf[:, :],
                                    pt[(h, t // 2)][:, t % 2, :],
                                    start=(t == 0), stop=(t == NKT - 1),
                                    tile_position=(0, 32 * h),
                                )

                        rden = op.tile([C, QT], FP32, tag="rden", name=f"rden{i}_{r}")
                        nc.vector.reciprocal_approx_fast(rden[:, :], den[:, :])
                        onorm = op.tile([C, QT], FP32R, tag="onorm", name=f"onorm{i}_{r}")
                        nc.vector.tensor_tensor(onorm[:, :], av[:, :], rden[:, :], AOP.mult)
                        nc.vector.tensor_scalar(
                            onorm[:, :], onorm[:, :], bias_v[:, :], None, AOP.add
                        )
                        wp_ps = ps_wp.tile([C, QT], FP32, tag="wp", name=f"wp{i}_{r}")
                        nc.tensor.matmul(
                            wp_ps[:, :], wp_r[:, :], onorm[:, :], start=True, stop=True
                        )
                        fin = op.tile([C, QT], FP32, tag="fin", name=f"fin{i}_{r}")
                        nc.vector.tensor_scalar(
                            fin[:, :], wp_ps[:, :], bp1[:, :], None, AOP.add
                        )
                        nc.sync.dma_start(out=out_d[:, i * QT : (i + 1) * QT], in_=fin)

    nc.finalize()
    return nc


def _get_nc():
    if "nc" not in _CACHE:
        _CACHE["nc"] = _build()
    return _CACHE["nc"]


def make_in_maps(inp):
    gmat = np.zeros((C, GROUPS), np.float32)
    gmat[np.arange(C), np.arange(C) // GSZ] = 1.0
    gtmat = np.ascontiguousarray(gmat.T)

    shared = {
        "wqt": np.ascontiguousarray(inp["Wq"].T * SCALE),
        "wkt": np.ascontiguousarray(inp["Wk"].T),
        "wvt": np.ascontiguousarray(inp["Wv"].T),
        "wpt": np.ascontiguousarray(inp["Wp"].T),
        "bqv": (inp["bq"] * SCALE).reshape(C, 1),
        "bkv": inp["bk"].reshape(C, 1),
        "bvv": inp["bv"].reshape(C, 1),
        "bpv": inp["bp"].reshape(C, 1),
        "gnq": inp["g_nq"].reshape(C, 1),
        "bnq": inp["b_nq"].reshape(C, 1),
        "gnk": inp["g_nk"].reshape(C, 1),
        "bnk": inp["b_nk"].reshape(C, 1),
        "gnv": inp["g_nv"].reshape(C, 1),
        "bnv": inp["b_nv"].reshape(C, 1),
        "gsrk": inp["g_srk"].reshape(C, 1),
        "bsrk": inp["b_srk"].reshape(C, 1),
        "gsrv": inp["g_srv"].reshape(C, 1),
        "bsrv": inp["b_srv"].reshape(C, 1),
        "gmat": gmat,
        "gtmat": gtmat,
    }
    shared = {k: np.ascontiguousarray(v, dtype=np.float32) for k, v in shared.items()}

    in_maps = []
    for c in range(B):
        m = dict(shared)
        m["qs"] = np.ascontiguousarray(inp["q_src"][c].reshape(C, S))
        m["ks"] = np.ascontiguousarray(inp["k_src"][c].reshape(C, S))
        m["vs"] = np.ascontiguousarray(inp["v_src"][c].reshape(C, S))
        in_maps.append(m)
    return in_maps


def kernel(**inputs) -> np.ndarray:
    inp = {k: np.asarray(v, dtype=np.float32) for k, v in inputs.items()}
    in_maps = make_in_maps(inp)
    nc = _get_nc()
    res = run_bass_kernel_spmd(nc, in_maps, core_ids=list(range(B)))
    out = np.stack([r["out"].reshape(C, H, W) for r in res.results], axis=0)
    return out

